# revision 1
# baseline (speedup 1.0000x reference)
"""Trainium2 Bass kernel for nn_DepthMemoryCache.

Reference computation (D=8, B=4, S=4096, C=1024, G=64):
    u     = einsum('bsc,gc->bsg', x[-1], W_u)
    keys  = einsum('dbc,gc->dbg', x.mean(2), W_u)
    gates = softmax(einsum('bsg,dbg->bsd', u, keys), axis=-1)
    out   = einsum('dbsc,bsd->bsc', x, gates)

Strategy: shard the sequence axis over 8 cores (core i gets
x[:, :, i*512:(i+1)*512, :]). Per core, two streaming passes over the 64MB
shard:
  A) depth/batch sums over s on PE: slabs are cast to bf16 (on the otherwise
     idle DVE/ACT engines) and column-summed with indicator stationaries in a
     single PSUM accumulation region. For the d=D-1 slabs, uT = W_u @ x7.T is
     also computed on PE (bf16 transposes + matmuls) so phase B needs no
     per-block transposes. A 128KB all-core AllReduce completes the
     full-sequence means (a tiny warm-up AllReduce at kernel start absorbs
     comm setup under phase A; collective bounce DMAs ride GpSimd's queue so
     the Sync engine keeps issuing prefetch reads).
  B) after a short fixup (meanT transposes + keysT matmuls), each 128-row
     block needs ONE small matmul for logits, softmax via ACT exp with
     accum_out, then 8 streamed depth tiles combined by fused
     scalar_tensor_tensor FMAs (fp32, exact) with per-partition gate scalars
     on DVE; gates are interleaved with streaming so the first FMA fires
     right after the collective.
HBM traffic per core: 64 (A) + 64 (B) + 8 (write) = 136MB.
The bf16 mean/logit paths cost ~1e-3/2e-4 relative on gates only; the output
weighted sum stays fp32.
"""
import sys

sys.path.insert(0, "/opt/trn_rl_repo")

from contextlib import ExitStack

import numpy as np
from concourse import bacc, bass, mybir, tile, masks
from concourse import bass_utils

F32 = mybir.dt.float32
BF16 = mybir.dt.bfloat16

D, B, S, C, G = 8, 4, 4096, 1024, 64
N_CORES = 8
P = 128                 # partition count / block rows
NKC = C // P            # 8 column chunks of 128


def build_body(tc, x, w, y, s_sh):
    """Emit the kernel IR. x:[D,B,s_sh,C], w:[G,C], y:[B,s_sh,C] dram APs."""
    nc = tc.nc
    nj = s_sh // P      # 128-row blocks per (d, b)
    mul, add = mybir.AluOpType.mult, mybir.AluOpType.add
    DB = D * B
    es = ExitStack()

    singles = es.enter_context(tc.tile_pool(name="singles", bufs=1))
    ident = singles.tile([P, P], F32)
    masks.make_identity(nc, ident[:])
    ident_bf = singles.tile([P, P], BF16)
    masks.make_identity(nc, ident_bf[:])
    # indicator stationaries: ind[:, r, m] = (m == r) / S  — column-sums a
    # bf16 slab into psum row r with one N=512 matmul per c-half.
    ind_bf = singles.tile([P, DB, DB], BF16)
    nc.vector.memset(ind_bf[:], 0.0)
    for r in range(DB):
        nc.vector.memset(ind_bf[:, r, r:r + 1], 1.0 / (N_CORES * s_sh))
    w_sb = singles.tile([G, C], F32)
    nc.sync.dma_start(w_sb[:], w[:])
    x7bf_sb = singles.tile([P, B, nj, C], BF16)
    gates_sb = singles.tile([P, B, nj, D], F32)
    sums_sb = singles.tile([DB, C], F32)
    sumk_sb = singles.tile([G, B * D], F32)
    meanT_sb = singles.tile([P, NKC * DB], F32)
    wT_sb = singles.tile([P, NKC, G], F32)
    wT_bf = singles.tile([P, NKC, G], BF16)
    keysT_sb = singles.tile([G, B, D], F32)
    uT_sb = singles.tile([G, B, nj, P], F32)

    stream = es.enter_context(tc.tile_pool(name="stream", bufs=3))
    bfp = es.enter_context(tc.tile_pool(name="bfp", bufs=2))

    dram = es.enter_context(tc.tile_pool(name="dram", bufs=1, space="DRAM"))
    # tiny warm-up AllReduce: absorbs collective-comm setup under phase A
    ccw_in = dram.tile([1, 16], F32)
    ccw_out = dram.tile([1, 16], F32)
    cc_in = dram.tile([G, B * D], F32)
    cc_out = dram.tile([G, B * D], F32)
    warm_sb = singles.tile([1, 16], F32)
    nc.vector.memset(warm_sb[:], 0.0)
    nc.gpsimd.dma_start(ccw_in[:], warm_sb[:])
    nc.gpsimd.collective_compute(
        "AllReduce", add, replica_groups=[list(range(N_CORES))],
        ins=[ccw_in.opt()], outs=[ccw_out.opt()],
    )

    # ---------------- Phase A: partial sums over s (scaled by 1/S) ----------
    with tc.tile_pool(name="psumA", bufs=1, space="PSUM") as psA, \
         tc.tile_pool(name="psumT", bufs=1, space="PSUM") as psT, \
         tc.tile_pool(name="psumXA", bufs=3, space="PSUM") as psXA, \
         tc.tile_pool(name="psumU", bufs=2, space="PSUM") as psU, \
         tc.tile_pool(name="xtA", bufs=3) as xtA:
        sums_ps = psA.tile([DB, C], F32)

        # Each 512-col half of sums_ps is one 2KB PSUM zero region: start=True
        # zeroes the WHOLE region, so exactly one start (global first MM into
        # that region) / one stop (global last); every other matmul
        # accumulates onto pending-zero bytes. Rows m != r get +0.
        def sum_slab(slab_bf, d, b, first, last):
            r = d * B + b
            for h in range(2):
                for j in range(nj):
                    nc.tensor.matmul(
                        sums_ps[:, h * 512:(h + 1) * 512],
                        ind_bf[:, r, :],
                        slab_bf[:, j, h * 512:(h + 1) * 512],
                        start=(first and j == 0),
                        stop=(last and j == nj - 1),
                    )

        def cast_slab(dst_bf, src_f32, i):
            # split the fp32->bf16 casts between DVE and ACT (both idle here)
            for j in range(nj):
                if (i * nj + j) % 2 == 0:
                    nc.vector.tensor_copy(dst_bf[:, j, :], src_f32[:, j, :])
                else:
                    nc.scalar.copy(dst_bf[:, j, :], src_f32[:, j, :])

        # one-time W_u transpose: wT[c, g] chunks (fp32 + bf16 copies)
        for k in range(NKC):
            tr = psT.tile([P, NKC * DB], F32, tag="fix")
            nc.tensor.transpose(tr[:, :G], w_sb[:, k * P:(k + 1) * P], ident[:G, :G])
            nc.vector.tensor_copy(wT_sb[:, k, :], tr[:, :G])
            nc.scalar.copy(wT_bf[:, k, :], tr[:, :G])

        def u_block(b, j):
            # uT[g, s-block] = sum_k (wT_k).T @ x7T_k on PE (reads resident
            # x7bf, so this can run any time after the d=D-1 cast)
            u_ps = psU.tile([G, P], F32, tag="u")
            for k in range(NKC):
                xt_ps = psXA.tile([P, P], BF16, tag="xt_ps")
                nc.tensor.transpose(
                    xt_ps[:], x7bf_sb[:, b, j, k * P:(k + 1) * P],
                    ident_bf[:])
                xt_sb = xtA.tile([P, P], BF16, tag="xt_sb")
                if k % 2 == 0:
                    nc.scalar.copy(xt_sb[:], xt_ps[:])
                else:
                    nc.vector.tensor_copy(xt_sb[:], xt_ps[:])
                nc.tensor.matmul(
                    u_ps[:], wT_bf[:, k, :], xt_sb[:],
                    start=(k == 0), stop=(k == NKC - 1))
            nc.vector.tensor_copy(uT_sb[:, b, j, :], u_ps[:])

        # d = 7 first (fills the resident x7bf); one uT block is interleaved
        # after every later slab so the PE/copy work spreads over phase A
        ublocks = [(b, j) for b in range(B) for j in range(nj)]
        ub_i = 0
        for dd in range(D):
            d = (dd + D - 1) % D
            for b in range(B):
                slab = stream.tile([P, nj, C], F32, tag="slab")
                nc.sync.dma_start(
                    slab[:], x[d, b].rearrange("(j p) c -> p j c", p=P))
                if d == D - 1:
                    xbf = x7bf_sb[:, b]
                else:
                    xbf_t = bfp.tile([P, nj, C], BF16, tag="xbf")
                    xbf = xbf_t[:]
                cast_slab(xbf, slab[:], d * B + b)
                sum_slab(xbf, d, b, first=(dd == 0 and b == 0),
                         last=(dd == D - 1 and b == B - 1))
                if dd >= 1 and ub_i < len(ublocks):
                    ub, uj = ublocks[ub_i]; ub_i += 1
                    u_block(ub, uj)
        while ub_i < len(ublocks):
            ub, uj = ublocks[ub_i]; ub_i += 1
            u_block(ub, uj)

        nc.vector.tensor_copy(sums_sb[:], sums_ps[:])

        # ---- local partial keysT (keys are linear in the means, so the ----
        # ---- AllReduce can run in the tiny keys space: 8KB not 128KB)  ----
        # meanT[c, (d,b)] chunks via PE transpose — all 8 into one psum tile
        # (one zero region => single start/stop accumulation group)
        mt_ps = psT.tile([P, NKC * DB], F32, tag="fix")
        for k in range(NKC):
            nc.tensor.matmul(
                mt_ps[:, k * DB:(k + 1) * DB],
                sums_sb[:, k * P:(k + 1) * P], ident[:DB, :DB],
                is_transpose=True, start=(k == 0), stop=(k == NKC - 1))
        nc.vector.tensor_copy(meanT_sb[:], mt_ps[:])
        # partial keysT[g, d] per b = sum_k wT_k.T @ meanT_k
        keys_ps = psT.tile([P, NKC * DB], F32, tag="fix")
        for b in range(B):
            for k in range(NKC):
                nc.tensor.matmul(
                    keys_ps[:G, b * D:(b + 1) * D],
                    wT_sb[:, k, :],
                    meanT_sb[:, k * DB:(k + 1) * DB].rearrange(
                        "p (d b) -> p d b", b=B)[:, :, b],
                    start=(k == 0), stop=(k == NKC - 1),
                )
        nc.vector.tensor_copy(sumk_sb[:], keys_ps[:G, :B * D])

    # ---------------- AllReduce the [G, B*D] partial keys -------------------
    # bounce DMAs go through GpSimd's queue so the Sync engine never blocks
    # on the collective and keeps issuing phase-B prefetch reads.
    nc.gpsimd.dma_start(cc_in[:], sumk_sb[:])
    nc.gpsimd.collective_compute(
        "AllReduce", add,
        replica_groups=[list(range(N_CORES))],
        ins=[cc_in.opt()], outs=[cc_out.opt()],
    )
    nc.gpsimd.dma_start(
        keysT_sb[:].rearrange("g b d -> g (b d)"), cc_out[:])

    # ---------------- Phase B: gates + depth-weighted sum -------------------
    with tc.tile_pool(name="psumL", bufs=2, space="PSUM") as psL, \
         tc.tile_pool(name="bstream", bufs=14) as bstream, \
         tc.tile_pool(name="accp", bufs=4) as accp, \
         tc.tile_pool(name="small", bufs=4) as small:
        for b in range(B):
            for j in range(nj):
                # logits for this block: one small matmul off resident uT
                lg_ps = psL.tile([P, D], F32, tag="lg")
                nc.tensor.matmul(lg_ps[:], uT_sb[:, b, j, :], keysT_sb[:, b, :])
                e_sb = small.tile([P, D], F32, tag="e")
                z_sb = small.tile([P, 1], F32, tag="z")
                rz_sb = small.tile([P, 1], F32, tag="rz")
                nc.scalar.activation(
                    e_sb[:], lg_ps[:], mybir.ActivationFunctionType.Exp,
                    accum_out=z_sb[:])
                nc.vector.reciprocal(rz_sb[:], z_sb[:])
                nc.scalar.mul(gates_sb[:, b, j, :], e_sb[:], rz_sb[:])

                acc = accp.tile([P, C], F32, tag="acc")
                for dd in range(D):
                    d = (dd + D - 1) % D        # d = 7 first, then 0..6
                    t = bstream.tile([P, C], F32, tag="bslab")
                    nc.sync.dma_start(
                        t[:], x[d, b, j * P:(j + 1) * P, :])
                    if dd == 0:
                        nc.vector.tensor_scalar_mul(
                            acc[:], t[:], gates_sb[:, b, j, d:d + 1])
                    else:
                        nc.vector.scalar_tensor_tensor(
                            out=acc[:], in0=t[:],
                            scalar=gates_sb[:, b, j, d:d + 1],
                            in1=acc[:], op0=mul, op1=add)
                # y writes via GpSimd (SWDGE): keeps both Sync's and ACT's
                # in-order queues free for prefetch reads / gate math
                nc.gpsimd.dma_start(y[b, j * P:(j + 1) * P, :], acc[:])

    es.close()


def build_nc(s_sh):
    nc = bacc.Bacc("TRN2", target_bir_lowering=False, debug=False,
                   num_devices=N_CORES)
    x_ap = nc.dram_tensor("x", [D, B, s_sh, C], F32, kind="ExternalInput").ap()
    w_ap = nc.dram_tensor("w", [G, C], F32, kind="ExternalInput").ap()
    y_ap = nc.dram_tensor("y", [B, s_sh, C], F32, kind="ExternalOutput").ap()
    with tile.TileContext(nc) as tc:
        build_body(tc, x_ap, w_ap, y_ap, s_sh)
    nc.compile()
    return nc


_NC_CACHE = {}


def _get_nc(s_sh):
    if s_sh not in _NC_CACHE:
        _NC_CACHE[s_sh] = build_nc(s_sh)
    return _NC_CACHE[s_sh]


def run(cached_states, W_u, trace=False, trace_cores=None):
    s_sh = S // N_CORES
    nc = _get_nc(s_sh)
    xs = np.asarray(cached_states, dtype=np.float32)
    ws = np.ascontiguousarray(np.asarray(W_u, dtype=np.float32))
    in_maps = []
    for i in range(N_CORES):
        sh = np.ascontiguousarray(xs[:, :, i * s_sh:(i + 1) * s_sh, :])
        in_maps.append({"x": sh, "w": ws})
    res = bass_utils.run_bass_kernel_spmd(
        nc, in_maps, core_ids=list(range(N_CORES)), trace=trace,
        trace_cores=trace_cores)
    out = np.empty((B, S, C), np.float32)
    for i in range(N_CORES):
        out[:, i * s_sh:(i + 1) * s_sh, :] = res.results[i]["y"]
    return out, res


def kernel(cached_states, W_u):
    out, _ = run(cached_states, W_u)
    return out



# revision 4
# speedup vs baseline: 1.1958x; 1.1958x over previous
"""Trainium2 Bass kernel for nn_DepthMemoryCache.

Reference computation (D=8, B=4, S=4096, C=1024, G=64):
    u     = einsum('bsc,gc->bsg', x[-1], W_u)
    keys  = einsum('dbc,gc->dbg', x.mean(2), W_u)
    gates = softmax(einsum('bsg,dbg->bsd', u, keys), axis=-1)
    out   = einsum('dbsc,bsd->bsc', x, gates)

Strategy: shard the sequence axis over 8 cores (core i gets
x[:, :, i*512:(i+1)*512, :]). Per core:

Phase A streams the 64MB shard once as 128 [128,1024] tiles through a
single SBUF ring. Each tile is cast to fp8-e4m3 (DVE/ACT alternating)
and column-summed by ONE DoubleRow matmul (K=256: the two c-halves ride
the two k-tiles, an indicator stationary routes half h of slab (d,b)
into psum row 2*(dB+b)+h), so the PE streams the whole shard at the
double-pumped fp8 rate. Depths 5,6,7 are additionally cast to resident
bf16 SBUF slabs. uT = W_u @ x7.T is computed on PE from the resident
bf16 x7 (transpose + matmul per c-chunk), interleaved across phase A.
The fixup (sums transpose -> keysT partials) runs in bf16, then an 8KB
AllReduce completes keys (a warm-up AllReduce at kernel start absorbs
the inter-core start skew; collective bounce DMAs ride GpSimd's queue).

Phase B shares the SAME tile ring: its per-block reads of the 5
non-resident depths queue directly behind phase A's reads in the DMA
rings, so the rings never drain across the collective. Per 128-row
block: one small logits matmul off resident uT, softmax via ACT exp
with accum_out + DVE reciprocal, then the depth-weighted sum as DVE
FMAs - resident depths first (bf16 sources), streamed depths (fp32) as
they land. y writes ride GpSimd (SWDGE).

HBM traffic per core: 64 (A) + 40 (B: 5/8 depths) + 8 (write) = 112MB.
fp8 only touches the gate path (means/logits ~2% -> gates ~2e-3);
resident bf16 touches 3/8 of the output sum (~2e-3); streamed depths
and the accumulator stay fp32.
"""
import sys

sys.path.insert(0, "/opt/trn_rl_repo")

from contextlib import ExitStack

import numpy as np
from concourse import bacc, bass, mybir, tile, masks
from concourse import bass_utils

F32 = mybir.dt.float32
BF16 = mybir.dt.bfloat16
F8 = mybir.dt.float8e4

D, B, S, C, G = 8, 4, 4096, 1024, 64
N_CORES = 8
P = 128                 # partition count / block rows
NKC = C // P            # 8 column chunks of 128
H = C // 2              # 512: c-half width (DoubleRow k-tile)
RESID = (7, 6, 5)       # depths resident in SBUF as bf16
POOL_BUFS = 12          # unified streaming ring


def build_body(tc, x, w, y, s_sh):
    """Emit the kernel IR. x:[D,B,s_sh,C], w:[G,C], y:[B,s_sh,C] dram APs."""
    nc = tc.nc
    nj = s_sh // P      # 128-row blocks per (d, b)
    mul, add = mybir.AluOpType.mult, mybir.AluOpType.add
    DB = D * B
    STREAMED = tuple(d for d in range(D) if d not in RESID)
    es = ExitStack()

    singles = es.enter_context(tc.tile_pool(name="singles", bufs=1))
    ident = singles.tile([P, P], F32)
    masks.make_identity(nc, ident[:])
    ident_bf = singles.tile([P, P], BF16)
    masks.make_identity(nc, ident_bf[:])
    # DoubleRow indicator: for slab r=(d*B+b), stationary ind_f8[:, r] is
    # [128, 2, 2*DB] with k-tile i routing c-half i into psum row 2r+i.
    ind_f8 = singles.tile([P, DB, 2, 2 * DB], F8)
    nc.vector.memset(ind_f8[:], 0.0)
    for r in range(DB):
        for i in range(2):
            nc.vector.memset(ind_f8[:, r, i, 2 * r + i:2 * r + i + 1], 1.0)
    w_sb = singles.tile([G, C], F32)
    nc.sync.dma_start(w_sb[:], w[:])
    xr = {}
    for d in RESID:
        xr[d] = singles.tile([P, B, nj, C], BF16, name=f"xr{d}")
    gates_sb = singles.tile([P, B, nj, D], F32)
    sums_sb = singles.tile([2 * DB, H], F32)        # row 2*(dB+b)+h
    sumk_sb = singles.tile([G, B * D], F32)
    meanT_bf = singles.tile([P, (NKC // 2) * 2 * DB], BF16)
    wT_bf = singles.tile([P, NKC, G], BF16)
    keysT_sb = singles.tile([G, B, D], F32)
    uT_sb = singles.tile([G, B, nj, P], F32)

    # unified streaming ring: phase-A reads and phase-B reads share it, so
    # phase-B prefetch queues directly behind phase A in the DMA rings
    pool = es.enter_context(tc.tile_pool(name="pool", bufs=POOL_BUFS))
    f8p = es.enter_context(tc.tile_pool(name="f8p", bufs=3))
    accp = es.enter_context(tc.tile_pool(name="accp", bufs=3))
    small = es.enter_context(tc.tile_pool(name="small", bufs=4))

    dram = es.enter_context(tc.tile_pool(name="dram", bufs=1, space="DRAM"))
    # tiny warm-up AllReduce: absorbs collective-comm setup under phase A
    ccw_in = dram.tile([1, 16], F32)
    ccw_out = dram.tile([1, 16], F32)
    cc_in = dram.tile([G, B * D], F32)
    cc_out = dram.tile([G, B * D], F32)
    warm_sb = singles.tile([1, 16], F32)
    nc.vector.memset(warm_sb[:], 0.0)
    nc.gpsimd.dma_start(ccw_in[:], warm_sb[:])
    nc.gpsimd.collective_compute(
        "AllReduce", add, replica_groups=[list(range(N_CORES))],
        ins=[ccw_in.opt()], outs=[ccw_out.opt()],
    )

    # ---------------- Phase A: stream + fp8 DoubleRow sums ------------------
    with tc.tile_pool(name="psumS", bufs=1, space="PSUM") as psS, \
         tc.tile_pool(name="psumT", bufs=1, space="PSUM") as psT, \
         tc.tile_pool(name="psumXA", bufs=3, space="PSUM") as psXA, \
         tc.tile_pool(name="psumU", bufs=2, space="PSUM") as psU, \
         tc.tile_pool(name="xtA", bufs=3) as xtA:
        sums_ps = psS.tile([2 * DB, H], F32)

        # one-time W_u transpose: wT[c, g] chunks in bf16
        for k in range(NKC):
            tr = psT.tile([P, 2 * DB * (NKC // 2)], F32, tag="fix")
            nc.tensor.transpose(tr[:, :G], w_sb[:, k * P:(k + 1) * P],
                                ident[:G, :G])
            nc.scalar.copy(wT_bf[:, k, :], tr[:, :G])

        def u_block(b, j):
            # uT[g, s-block] = sum_k (wT_k).T @ x7T_k on PE (reads resident
            # bf16 x7, so this can run any time after the d=7 casts)
            u_ps = psU.tile([G, P], F32, tag="u")
            for k in range(NKC):
                xt_ps = psXA.tile([P, P], BF16, tag="xt_ps")
                nc.tensor.transpose(
                    xt_ps[:], xr[D - 1][:, b, j, k * P:(k + 1) * P],
                    ident_bf[:])
                xt_sb = xtA.tile([P, P], BF16, tag="xt_sb")
                if k % 2 == 0:
                    nc.scalar.copy(xt_sb[:], xt_ps[:])
                else:
                    nc.vector.tensor_copy(xt_sb[:], xt_ps[:])
                nc.tensor.matmul(
                    u_ps[:], wT_bf[:, k, :], xt_sb[:],
                    start=(k == 0), stop=(k == NKC - 1))
            nc.vector.tensor_copy(uT_sb[:, b, j, :], u_ps[:])

        ublocks = [(b, j) for b in range(B) for j in range(nj)]
        ub_i = 0
        ti = 0
        NT = DB * nj
        for dd in range(D):
            d = (dd + D - 1) % D        # d = 7 first (fills resident x7)
            for b in range(B):
                r = d * B + b
                for j in range(nj):
                    t = pool.tile([P, C], F32, tag="t")
                    nc.sync.dma_start(t[:], x[d, b, j * P:(j + 1) * P, :])
                    q = f8p.tile([P, C], F8, tag="q")
                    if ti % 2 == 0:
                        nc.vector.tensor_copy(q[:], t[:])
                        if d in RESID:
                            nc.scalar.copy(xr[d][:, b, j, :], t[:])
                    else:
                        nc.scalar.copy(q[:], t[:])
                        if d in RESID:
                            nc.vector.tensor_copy(xr[d][:, b, j, :], t[:])
                    nc.tensor.matmul(
                        sums_ps[:], ind_f8[:, r],
                        q[:].rearrange("p (i h) -> p i h", i=2),
                        start=(ti == 0), stop=(ti == NT - 1),
                        perf_mode=mybir.MatmulPerfMode.DoubleRow)
                    if dd >= 1 and ti % 4 == 0 and ub_i < len(ublocks):
                        ub, uj = ublocks[ub_i]
                        ub_i += 1
                        u_block(ub, uj)
                    ti += 1
        while ub_i < len(ublocks):
            ub, uj = ublocks[ub_i]
            ub_i += 1
            u_block(ub, uj)

        # raw sums -> sbuf with the 1/S mean scale folded in (ACT)
        nc.scalar.mul(sums_sb[:], sums_ps[:], 1.0 / S)

        # ---- local partial keysT in bf16 (keys are linear in the means, ----
        # ---- so the AllReduce runs in the tiny keys space: 8KB)         ----
        # meanT chunks: 4 fp32 transposes [64,128] -> [128,64] into one psum
        mt_ps = psT.tile([P, 2 * DB * (NKC // 2)], F32, tag="fix")
        for kp in range(NKC // 2):
            nc.tensor.matmul(
                mt_ps[:, kp * 2 * DB:(kp + 1) * 2 * DB],
                sums_sb[:, kp * P:(kp + 1) * P], ident[:2 * DB, :2 * DB],
                is_transpose=True,
                start=(kp == 0), stop=(kp == NKC // 2 - 1))
        nc.vector.tensor_copy(meanT_bf[:], mt_ps[:])
        # partial keysT[g, d] per b = sum_(h,k') wT_(h*4+k').T @ meanT cols
        keys_ps = psT.tile([P, 2 * DB * (NKC // 2)], F32, tag="fix")
        for bb in range(B):
            for kk in range(NKC):
                h, kp = divmod(kk, NKC // 2)
                mcols = meanT_bf[:, kp * 2 * DB:(kp + 1) * 2 * DB].rearrange(
                    "p (dd m) -> p dd m", m=2 * B)[:, :, 2 * bb + h]
                nc.tensor.matmul(
                    keys_ps[:G, bb * D:(bb + 1) * D],
                    wT_bf[:, kk, :], mcols,
                    start=(kk == 0), stop=(kk == NKC - 1))
        nc.vector.tensor_copy(sumk_sb[:], keys_ps[:G, :B * D])

    # ---------------- AllReduce the [G, B*D] partial keys -------------------
    # bounce DMAs go through GpSimd's queue so the Sync engine never blocks
    # on the collective and keeps issuing phase-B prefetch reads.
    nc.gpsimd.dma_start(cc_in[:], sumk_sb[:])
    nc.gpsimd.collective_compute(
        "AllReduce", add,
        replica_groups=[list(range(N_CORES))],
        ins=[cc_in.opt()], outs=[cc_out.opt()],
    )
    nc.gpsimd.dma_start(
        keysT_sb[:].rearrange("g b d -> g (b d)"), cc_out[:])

    # ---------------- Phase B: gates + depth-weighted sum -------------------
    with tc.tile_pool(name="psumL", bufs=2, space="PSUM") as psL:
        for b in range(B):
            for j in range(nj):
                ts = {}
                for sd in STREAMED:
                    tt = pool.tile([P, C], F32, tag="t")
                    nc.sync.dma_start(tt[:], x[sd, b, j * P:(j + 1) * P, :])
                    ts[sd] = tt
                # logits for this block: one small matmul off resident uT
                lg_ps = psL.tile([P, D], F32, tag="lg")
                nc.tensor.matmul(lg_ps[:], uT_sb[:, b, j, :],
                                 keysT_sb[:, b, :])
                e_sb = small.tile([P, D], F32, tag="e")
                z_sb = small.tile([P, 1], F32, tag="z")
                rz_sb = small.tile([P, 1], F32, tag="rz")
                nc.scalar.activation(
                    e_sb[:], lg_ps[:], mybir.ActivationFunctionType.Exp,
                    accum_out=z_sb[:])
                nc.vector.reciprocal(rz_sb[:], z_sb[:])
                nc.scalar.mul(gates_sb[:, b, j, :], e_sb[:], rz_sb[:])

                acc = accp.tile([P, C], F32, tag="acc")
                first = True
                for d in RESID:
                    g = gates_sb[:, b, j, d:d + 1]
                    if first:
                        nc.vector.tensor_scalar_mul(
                            acc[:], xr[d][:, b, j, :], g)
                        first = False
                    else:
                        nc.vector.scalar_tensor_tensor(
                            out=acc[:], in0=xr[d][:, b, j, :], scalar=g,
                            in1=acc[:], op0=mul, op1=add)
                for d in STREAMED:
                    nc.vector.scalar_tensor_tensor(
                        out=acc[:], in0=ts[d][:],
                        scalar=gates_sb[:, b, j, d:d + 1],
                        in1=acc[:], op0=mul, op1=add)
                # y writes via GpSimd (SWDGE): keeps Sync's in-order queue
                # free for prefetch reads
                nc.gpsimd.dma_start(y[b, j * P:(j + 1) * P, :], acc[:])

    es.close()


def build_nc(s_sh):
    nc = bacc.Bacc("TRN2", target_bir_lowering=False, debug=False,
                   num_devices=N_CORES)
    x_ap = nc.dram_tensor("x", [D, B, s_sh, C], F32, kind="ExternalInput").ap()
    w_ap = nc.dram_tensor("w", [G, C], F32, kind="ExternalInput").ap()
    y_ap = nc.dram_tensor("y", [B, s_sh, C], F32, kind="ExternalOutput").ap()
    with tile.TileContext(nc) as tc:
        build_body(tc, x_ap, w_ap, y_ap, s_sh)
    nc.compile()
    return nc


_NC_CACHE = {}


def _get_nc(s_sh):
    if s_sh not in _NC_CACHE:
        _NC_CACHE[s_sh] = build_nc(s_sh)
    return _NC_CACHE[s_sh]


def run(cached_states, W_u, trace=False, trace_cores=None):
    s_sh = S // N_CORES
    nc = _get_nc(s_sh)
    xs = np.asarray(cached_states, dtype=np.float32)
    ws = np.ascontiguousarray(np.asarray(W_u, dtype=np.float32))
    in_maps = []
    for i in range(N_CORES):
        sh = np.ascontiguousarray(xs[:, :, i * s_sh:(i + 1) * s_sh, :])
        in_maps.append({"x": sh, "w": ws})
    res = bass_utils.run_bass_kernel_spmd(
        nc, in_maps, core_ids=list(range(N_CORES)), trace=trace,
        trace_cores=trace_cores)
    out = np.empty((B, S, C), np.float32)
    for i in range(N_CORES):
        out[:, i * s_sh:(i + 1) * s_sh, :] = res.results[i]["y"]
    return out, res


def kernel(cached_states, W_u):
    out, _ = run(cached_states, W_u)
    return out
